# revision 23
# baseline (speedup 1.0000x reference)
"""Trainium2 Bass kernel for nn_ConceptIntergation (histogram_binning).

Reference computation:
    counts[b,s,n] = sum_k one_hot(concepts[b,s,k], 129)[..., n]  (n < 128; 128 = padding)
    out[b,s,n,d]  = counts[b,s,n] * emb_table[n,d]

Strategy (data-parallel over batch, 8 cores; DMA-write floor ~123us/core at
the ~427 GB/s measured sustained store rate):
  - Each core handles B_LOC=8 batches -> 1600 (b,s) rows, output shard
    [1600, 128*64] f32 (~52 MB). All inputs arrive in ONE ~190KB DMA.
  - Dual-path expansion keeps every engine under the DMA floor (the PE
    here is pinned at 1.2GHz, ~512ns per 512-col matmul, so a pure-PE
    expansion would pace the store stream):
      * PE path (stripe 0 + ~half of stripes 1-3):
        out_block = countsT.T @ W with W[n', n*64+d] = emb[n,d]*(n==n')
        a block-diagonal [128, 8192] bf16 matrix built on-device by DVE
        tensor_tensor; countsT = PE transpose of the DVE histogram;
        PSUM->SBUF drains alternate ScalarE/VectorE.
      * DVE path (remaining chunks): direct broadcast multiply
        counts[:, n] * embrep[:, (n d)] where embrep = ones.T @ W
        (column sums of the block-diagonal W = the dense emb replica),
        16 one-time PE matmuls -- no HBM read, no PSUM traffic.
    bf16 appears only in W / countsT (exact small-int counts; emb rounds
    once, rel err <= 2^-8 < 2e-2), accumulated in f32 PSUM.
  - Approx engine busy/core: PE ~90us, DVE ~90us, ScalarE ~65us, all
    under the ~123us DMA floor; stripe 0 stores 0.5MB chunks so the
    first store issues ~6us after the inputs land.
"""

import numpy as np

import concourse.bass as bass
import concourse.mybir as mybir
from concourse.bass import _add_dep_helper
from concourse import bacc
from concourse.tile import TileContext
from concourse.bass_utils import run_bass_kernel_spmd

B, S, K = 64, 200, 4
N, D = 128, 64
ND = N * D                      # 8192
NCORES = 8
B_LOC = B // NCORES             # 8
ROWS = B_LOC * S                # 1600 (b,s) rows per core
P = 128
NBLK = (ROWS + P - 1) // P      # 13 (12 full + 1 of 64 rows)

CC = 4                          # W chunks / output column stripes
CW = ND // CC                   # 2048 cols per stripe
MW = CW // D                    # 32 n-rows per stripe
FD = 512                        # matmul moving free dim (1 PSUM bank f32)

# packed const input column offsets
C_IOTA = 0
C_IDX = C_IOTA + N              # 128
C_EMB = C_IDX + NBLK * K        # 180
C_ID = C_EMB + D                # 244
C_TOT = C_ID + P                # 372

F32 = mybir.dt.float32
BF16 = mybir.dt.bfloat16

# chunks routed to the direct-DVE path, per stripe (by position in the
# block emission order); stripe 0 is PE-only (embrep not built yet).
# Weighted toward the END: the PE (pinned at 1.2GHz) falls behind late in
# the kernel, and the TT->store chain also shortens the kernel tail.
DVE_SET = {
    0: set(),
    1: set(),
    2: set(),
    3: {6, 8, 10, 11, 12},
}
# embrep build units (q): only the stripe-3 section is needed; build it
# during stripe 1 where ScalarE/PE have slack
EXTRA0 = {}
EXTRA1 = {3: [(3, 0)], 5: [(3, 1)], 7: [(3, 2)], 9: [(3, 3)]}

_NC_CACHE = {}


def _build_nc():
    nc = bacc.Bacc()
    cst = nc.declare_dram_parameter("cst", [P, C_TOT], F32, isOutput=False)
    out = nc.declare_dram_parameter("out", [ROWS, ND], F32, isOutput=True)

    with TileContext(nc) as tc:
        with (
            tc.tile_pool(name="const", bufs=1) as cpool,
            tc.tile_pool(name="cnt", bufs=NBLK) as cntpool,
            tc.tile_pool(name="cntT", bufs=NBLK) as ctpool,
            tc.tile_pool(name="work", bufs=12) as wpool,
            tc.tile_pool(name="work1", bufs=8) as wpool1,
            tc.tile_pool(name="psmm", bufs=3, space="PSUM") as pmm,
            tc.tile_pool(name="pstr", bufs=2, space="PSUM") as ptr,
        ):
            cst_sb = cpool.tile([P, C_TOT], F32)
            nc.sync.dma_start(out=cst_sb, in_=cst[:, :])
            iota_sb = cst_sb[:, C_IOTA : C_IOTA + N]
            idx_sb = cst_sb[:, C_IDX : C_IDX + NBLK * K]
            emb_sb = cst_sb[:, C_EMB : C_EMB + D]
            ident_sb = cst_sb[:, C_ID : C_ID + P]

            ones_sb = cpool.tile([P, P], BF16)
            nc.vector.memset(ones_sb[:, :], 1.0)

            Wt = [
                cpool.tile([P, CW], BF16, tag=f"W{c}", name=f"W{c}")
                for c in range(CC)
            ]
            # dense emb replica: only the stripe-3 section is ever used
            Et3 = cpool.tile([P, CW], F32, tag="E3", name="E3")
            Et = {3: Et3}

            def build_w(c, lo, hi):
                # W[n, (m d)] = emb[n, d] * (n == c*MW + m) for m in [lo, hi)
                return nc.vector.tensor_tensor(
                    out=Wt[c][:, lo * D : hi * D].rearrange(
                        "p (m d) -> p m d", d=D
                    ),
                    in0=emb_sb[:, None, :].broadcast_to([P, hi - lo, D]),
                    in1=ident_sb[:, c * MW + lo : c * MW + hi, None].broadcast_to(
                        [P, hi - lo, D]
                    ),
                    op=mybir.AluOpType.mult,
                )

            def build_embrep(c, q):
                # column sums of block-diagonal W == dense emb replica
                pse = ptr.tile([P, FD], F32, tag="pst")
                nc.tensor.matmul(
                    pse[:, :],
                    ones_sb[:, :],
                    Wt[c][:, q * FD : (q + 1) * FD],
                    start=True,
                    stop=True,
                )
                nc.scalar.activation(
                    Et[c][:, q * FD : (q + 1) * FD],
                    pse[:, :],
                    mybir.ActivationFunctionType.Copy,
                )

            def emit_countsT(j, pj):
                counts = cntpool.tile([P, N], F32, tag="cnt")
                nc.vector.tensor_scalar(
                    out=counts[:pj],
                    in0=iota_sb[:pj],
                    scalar1=idx_sb[:pj, j * K : j * K + 1],
                    scalar2=None,
                    op0=mybir.AluOpType.is_equal,
                )
                for k in range(1, K):
                    state["hist_last"] = nc.vector.scalar_tensor_tensor(
                        out=counts[:pj],
                        in0=iota_sb[:pj],
                        scalar=idx_sb[:pj, j * K + k : j * K + k + 1],
                        in1=counts[:pj],
                        op0=mybir.AluOpType.is_equal,
                        op1=mybir.AluOpType.add,
                    )
                pst = ptr.tile([P, P], F32, tag="pst")
                nc.tensor.transpose(
                    pst[:, :pj], counts[:pj, :], ident_sb[:pj, :pj]
                )
                ct = ctpool.tile([P, P], BF16, tag="ct")
                nc.scalar.activation(
                    ct[:, :pj], pst[:, :pj], mybir.ActivationFunctionType.Copy
                )
                return counts, ct

            state = {"ncopy": 0}

            def drain_copy(dst, src):
                # ~60% on ScalarE: DVE also runs histograms, W builds and
                # the direct-multiply chunks
                if state["ncopy"] % 5 != 1 and state["ncopy"] % 5 != 3:
                    nc.scalar.activation(
                        dst, src, mybir.ActivationFunctionType.Copy
                    )
                else:
                    nc.vector.tensor_copy(out=dst, in_=src)
                state["ncopy"] += 1

            def emit_quarter(cc, j, pj, ct, q):
                """one [pj, FD] quarter-chunk: 1 matmul + 1 copy + 0.25MB store"""
                ps = pmm.tile([P, 2 * FD], F32, tag="ps")
                nc.tensor.matmul(
                    ps[:pj, :FD],
                    ct[:, :pj],
                    Wt[cc][:, q * FD : (q + 1) * FD],
                    start=True,
                    stop=True,
                )
                otq = wpool1.tile([P, 2 * FD], F32, tag="ot1")
                drain_copy(otq[:pj, :FD], ps[:pj, :FD])
                nc.sync.dma_start(
                    out=out[
                        j * P : j * P + pj,
                        cc * CW + q * FD : cc * CW + (q + 1) * FD,
                    ],
                    in_=otq[:pj, :FD],
                )

            def emit_half(cc, j, pj, ct, h, ot=None):
                """one [pj, 2*FD] half-chunk via PE: 2 matmuls + drain copy.
                Stores directly (0.5MB) if ot is None."""
                ps = pmm.tile([P, 2 * FD], F32, tag="ps")
                for q in range(2):
                    nc.tensor.matmul(
                        ps[:pj, q * FD : (q + 1) * FD],
                        ct[:, :pj],
                        Wt[cc][:, (2 * h + q) * FD : (2 * h + q + 1) * FD],
                        start=True,
                        stop=True,
                    )
                if ot is None:
                    ot1 = wpool1.tile([P, 2 * FD], F32, tag="ot1")
                    drain_copy(ot1[:pj], ps[:pj])
                    nc.sync.dma_start(
                        out=out[
                            j * P : j * P + pj,
                            cc * CW + 2 * h * FD : cc * CW + 2 * (h + 1) * FD,
                        ],
                        in_=ot1[:pj],
                    )
                else:
                    drain_copy(ot[:pj, 2 * h * FD : 2 * (h + 1) * FD], ps[:pj])

            # Partial block (64 rows) first so its half-width DMAs overlap
            # the full-width stream instead of trailing it.
            order = [NBLK - 1] + list(range(NBLK - 1))
            cnts = [None] * NBLK
            cts = [None] * NBLK
            for cc in range(CC):
                for oi, j in enumerate(order):
                    pj = min(P, ROWS - j * P)
                    if cc == 0:
                        cnts[j], cts[j] = emit_countsT(j, pj)
                        if oi == 0:
                            # order the W build after the first histogram on
                            # DVE: interleaving would inflate the first
                            # transpose's DVE-op-count semaphore (~3us ramp)
                            w0a = build_w(0, 0, MW // 2)
                            _add_dep_helper(
                                w0a.ins,
                                state["hist_last"].ins,
                                sync=False,
                                reason="keep first hist ops contiguous",
                            )
                            # first block streams out in 0.25MB quarters:
                            # shortest possible chain to the first store
                            for q in range(2):
                                emit_quarter(cc, j, pj, cts[j], q)
                            w0b = build_w(0, MW // 2, MW)
                            _add_dep_helper(
                                w0b.ins, w0a.ins, sync=False,
                                reason="W0 halves in order",
                            )
                            for q in range(2, 4):
                                emit_quarter(cc, j, pj, cts[j], q)
                            continue
                        if oi in (5, 8, 11):
                            build_w(oi // 3, 0, MW)
                        emit_half(cc, j, pj, cts[j], 0)
                        emit_half(cc, j, pj, cts[j], 1)
                        for c, q in EXTRA0.get(oi, ()):
                            build_embrep(c, q)
                    elif oi in DVE_SET[cc]:
                        # direct broadcast multiply on DVE, no PSUM
                        ot = wpool.tile([P, CW], F32, tag="ot")
                        nc.vector.tensor_tensor(
                            out=ot[:pj].rearrange("p (m d) -> p m d", d=D),
                            in0=cnts[j][
                                :pj, cc * MW : (cc + 1) * MW, None
                            ].broadcast_to([pj, MW, D]),
                            in1=Et[cc][:pj].rearrange("p (m d) -> p m d", d=D),
                            op=mybir.AluOpType.mult,
                        )
                        nc.sync.dma_start(
                            out=out[j * P : j * P + pj, cc * CW : (cc + 1) * CW],
                            in_=ot[:pj],
                        )
                    else:
                        ot = wpool.tile([P, CW], F32, tag="ot")
                        for h in range(2):
                            emit_half(cc, j, pj, cts[j], h, ot=ot)
                        nc.sync.dma_start(
                            out=out[j * P : j * P + pj, cc * CW : (cc + 1) * CW],
                            in_=ot[:pj],
                        )
                        if cc == 1:
                            for c, q in EXTRA1.get(oi, ()):
                                build_embrep(c, q)

    nc.finalize()
    return nc


def _get_nc():
    if "nc" not in _NC_CACHE:
        _NC_CACHE["nc"] = _build_nc()
    return _NC_CACHE["nc"]


def _prepare_in_maps(concepts, emb_table):
    concepts = np.asarray(concepts)
    emb = np.asarray(emb_table, dtype=np.float32)

    # per-core index shards, padded to NBLK*P rows, laid out [P, NBLK*K]
    conc = concepts.reshape(NCORES, ROWS, K).astype(np.float32)
    idx_pad = np.full((NCORES, NBLK * P, K), float(N), dtype=np.float32)
    idx_pad[:, :ROWS] = conc
    # [core, NBLK, P, K] -> [core, P, NBLK*K]
    idx_dev = idx_pad.reshape(NCORES, NBLK, P, K).transpose(0, 2, 1, 3).reshape(
        NCORES, P, NBLK * K
    )

    cst = np.empty((NCORES, P, C_TOT), dtype=np.float32)
    cst[:, :, C_IOTA : C_IOTA + N] = np.arange(N, dtype=np.float32)
    cst[:, :, C_IDX : C_IDX + NBLK * K] = idx_dev
    cst[:, :, C_EMB : C_EMB + D] = emb
    cst[:, :, C_ID : C_ID + P] = np.eye(P, dtype=np.float32)
    cst = np.ascontiguousarray(cst)
    return [{"cst": cst[i]} for i in range(NCORES)]


def _run(concepts, emb_table, **spmd_kwargs):
    nc = _get_nc()
    in_maps = _prepare_in_maps(concepts, emb_table)
    res = run_bass_kernel_spmd(nc, in_maps, core_ids=list(range(NCORES)), **spmd_kwargs)
    out = np.concatenate(
        [res.results[i]["out"].reshape(B_LOC, S, N, D) for i in range(NCORES)],
        axis=0,
    )
    return out, res


def kernel(concepts, emb_table):
    out, _ = _run(concepts, emb_table)
    return out


# revision 25
# speedup vs baseline: 1.0725x; 1.0725x over previous
"""Trainium2 Bass kernel for nn_ConceptIntergation (histogram_binning).

Reference computation:
    counts[b,s,n] = sum_k one_hot(concepts[b,s,k], 129)[..., n]  (n < 128; 128 = padding)
    out[b,s,n,d]  = counts[b,s,n] * emb_table[n,d]

Strategy (data-parallel over batch, 8 cores; DMA-write floor ~123us/core at
the ~427 GB/s measured sustained store rate):
  - Each core handles B_LOC=8 batches -> 1600 (b,s) rows, output shard
    [1600, 128*64] f32 (~52 MB). All inputs arrive in ONE ~190KB DMA.
  - Dual-path expansion keeps every engine under the DMA floor (the PE
    here is pinned at 1.2GHz, ~512ns per 512-col matmul, so a pure-PE
    expansion would pace the store stream):
      * PE path (stripe 0 + ~half of stripes 1-3):
        out_block = countsT.T @ W with W[n', n*64+d] = emb[n,d]*(n==n')
        a block-diagonal [128, 8192] bf16 matrix built on-device by DVE
        tensor_tensor; countsT = PE transpose of the DVE histogram;
        PSUM->SBUF drains alternate ScalarE/VectorE.
      * DVE path (remaining chunks): direct broadcast multiply
        counts[:, n] * embrep[:, (n d)] where embrep = ones.T @ W
        (column sums of the block-diagonal W = the dense emb replica),
        16 one-time PE matmuls -- no HBM read, no PSUM traffic.
    bf16 appears only in W / countsT (exact small-int counts; emb rounds
    once, rel err <= 2^-8 < 2e-2), accumulated in f32 PSUM.
  - Approx engine busy/core: PE ~90us, DVE ~90us, ScalarE ~65us, all
    under the ~123us DMA floor; stripe 0 stores 0.5MB chunks so the
    first store issues ~6us after the inputs land.
"""

import numpy as np

import concourse.bass as bass
import concourse.mybir as mybir
from concourse.bass import _add_dep_helper
from concourse import bacc
from concourse.tile import TileContext
from concourse.bass_utils import run_bass_kernel_spmd

B, S, K = 64, 200, 4
N, D = 128, 64
ND = N * D                      # 8192
NCORES = 8
B_LOC = B // NCORES             # 8
ROWS = B_LOC * S                # 1600 (b,s) rows per core
P = 128
NBLK = (ROWS + P - 1) // P      # 13 (12 full + 1 of 64 rows)

CC = 4                          # W chunks / output column stripes
CW = ND // CC                   # 2048 cols per stripe
MW = CW // D                    # 32 n-rows per stripe
FD = 512                        # matmul moving free dim (1 PSUM bank f32)

# packed const input column offsets
C_IOTA = 0
C_IDX = C_IOTA + N              # 128
C_EMB = C_IDX + NBLK * K        # 180
C_ID = C_EMB + D                # 244
C_TOT = C_ID + P                # 372

F32 = mybir.dt.float32
BF16 = mybir.dt.bfloat16

# chunks routed to the direct-DVE path, per stripe (by position in the
# block emission order); stripe 0 is PE-only (embrep not built yet).
# Weighted toward the END: the PE (pinned at 1.2GHz) falls behind late in
# the kernel, and the TT->store chain also shortens the kernel tail.
DVE_SET = {
    0: set(),
    1: set(),
    2: set(),
    3: {6, 8, 10, 11, 12},
}
# embrep build units (q): only the stripe-3 section is needed; build it
# during stripe 1 where ScalarE/PE have slack
EXTRA0 = {}
EXTRA1 = {3: [(3, 0)], 5: [(3, 1)], 7: [(3, 2)], 9: [(3, 3)]}

_NC_CACHE = {}


def _build_nc():
    nc = bacc.Bacc()
    cst = nc.declare_dram_parameter("cst", [P, C_TOT], F32, isOutput=False)
    out = nc.declare_dram_parameter("out", [ROWS, ND], F32, isOutput=True)

    with TileContext(nc) as tc:
        with (
            tc.tile_pool(name="const", bufs=1) as cpool,
            tc.tile_pool(name="cnt", bufs=NBLK) as cntpool,
            tc.tile_pool(name="cntT", bufs=NBLK) as ctpool,
            tc.tile_pool(name="work", bufs=10) as wpool,
            tc.tile_pool(name="work1", bufs=6) as wpool1,
            tc.tile_pool(name="psmm", bufs=3, space="PSUM") as pmm,
            tc.tile_pool(name="pstr", bufs=2, space="PSUM") as ptr,
        ):
            cst_sb = cpool.tile([P, C_TOT], F32)
            nc.sync.dma_start(out=cst_sb, in_=cst[:, :])
            iota_sb = cst_sb[:, C_IOTA : C_IOTA + N]
            idx_sb = cst_sb[:, C_IDX : C_IDX + NBLK * K]
            emb_sb = cst_sb[:, C_EMB : C_EMB + D]
            ident_sb = cst_sb[:, C_ID : C_ID + P]

            ones_sb = cpool.tile([P, P], BF16)
            nc.vector.memset(ones_sb[:, :], 1.0)

            Wt = [
                cpool.tile([P, CW], BF16, tag=f"W{c}", name=f"W{c}")
                for c in range(CC)
            ]
            # dense emb replica: only the stripe-3 section is ever used
            Et3 = cpool.tile([P, CW], F32, tag="E3", name="E3")
            Et = {3: Et3}

            def build_w(c, lo, hi):
                # W[n, (m d)] = emb[n, d] * (n == c*MW + m) for m in [lo, hi)
                return nc.vector.tensor_tensor(
                    out=Wt[c][:, lo * D : hi * D].rearrange(
                        "p (m d) -> p m d", d=D
                    ),
                    in0=emb_sb[:, None, :].broadcast_to([P, hi - lo, D]),
                    in1=ident_sb[:, c * MW + lo : c * MW + hi, None].broadcast_to(
                        [P, hi - lo, D]
                    ),
                    op=mybir.AluOpType.mult,
                )

            def build_embrep(c, q):
                # column sums of block-diagonal W == dense emb replica
                pse = ptr.tile([P, FD], F32, tag="pst")
                nc.tensor.matmul(
                    pse[:, :],
                    ones_sb[:, :],
                    Wt[c][:, q * FD : (q + 1) * FD],
                    start=True,
                    stop=True,
                )
                nc.scalar.activation(
                    Et[c][:, q * FD : (q + 1) * FD],
                    pse[:, :],
                    mybir.ActivationFunctionType.Copy,
                )

            def emit_countsT(j, pj):
                counts = cntpool.tile([P, N], F32, tag="cnt")
                nc.vector.tensor_scalar(
                    out=counts[:pj],
                    in0=iota_sb[:pj],
                    scalar1=idx_sb[:pj, j * K : j * K + 1],
                    scalar2=None,
                    op0=mybir.AluOpType.is_equal,
                )
                for k in range(1, K):
                    state["hist_last"] = nc.vector.scalar_tensor_tensor(
                        out=counts[:pj],
                        in0=iota_sb[:pj],
                        scalar=idx_sb[:pj, j * K + k : j * K + k + 1],
                        in1=counts[:pj],
                        op0=mybir.AluOpType.is_equal,
                        op1=mybir.AluOpType.add,
                    )
                pst = ptr.tile([P, P], F32, tag="pst")
                nc.tensor.transpose(
                    pst[:, :pj], counts[:pj, :], ident_sb[:pj, :pj]
                )
                ct = ctpool.tile([P, P], BF16, tag="ct")
                nc.scalar.activation(
                    ct[:, :pj], pst[:, :pj], mybir.ActivationFunctionType.Copy
                )
                return counts, ct

            state = {"ncopy": 0}

            def drain_copy(dst, src):
                # ~60% on ScalarE: DVE also runs histograms, W builds and
                # the direct-multiply chunks
                if state["ncopy"] % 5 != 1 and state["ncopy"] % 5 != 3:
                    nc.scalar.activation(
                        dst, src, mybir.ActivationFunctionType.Copy
                    )
                else:
                    nc.vector.tensor_copy(out=dst, in_=src)
                state["ncopy"] += 1

            def emit_quarter(cc, j, pj, ct, q):
                """one [pj, FD] quarter-chunk: 1 matmul + 1 copy + 0.25MB store"""
                ps = pmm.tile([P, 2 * FD], F32, tag="ps")
                nc.tensor.matmul(
                    ps[:pj, :FD],
                    ct[:, :pj],
                    Wt[cc][:, q * FD : (q + 1) * FD],
                    start=True,
                    stop=True,
                )
                otq = wpool1.tile([P, 2 * FD], F32, tag="ot1")
                drain_copy(otq[:pj, :FD], ps[:pj, :FD])
                nc.sync.dma_start(
                    out=out[
                        j * P : j * P + pj,
                        cc * CW + q * FD : cc * CW + (q + 1) * FD,
                    ],
                    in_=otq[:pj, :FD],
                )

            def emit_half(cc, j, pj, ct, h, ot=None):
                """one [pj, 2*FD] half-chunk via PE: 2 matmuls + drain copy.
                Stores directly (0.5MB) if ot is None."""
                ps = pmm.tile([P, 2 * FD], F32, tag="ps")
                for q in range(2):
                    nc.tensor.matmul(
                        ps[:pj, q * FD : (q + 1) * FD],
                        ct[:, :pj],
                        Wt[cc][:, (2 * h + q) * FD : (2 * h + q + 1) * FD],
                        start=True,
                        stop=True,
                    )
                if ot is None:
                    ot1 = wpool1.tile([P, 2 * FD], F32, tag="ot1")
                    drain_copy(ot1[:pj], ps[:pj])
                    nc.sync.dma_start(
                        out=out[
                            j * P : j * P + pj,
                            cc * CW + 2 * h * FD : cc * CW + 2 * (h + 1) * FD,
                        ],
                        in_=ot1[:pj],
                    )
                else:
                    drain_copy(ot[:pj, 2 * h * FD : 2 * (h + 1) * FD], ps[:pj])

            # Partial block (64 rows) first so its half-width DMAs overlap
            # the full-width stream instead of trailing it.
            order = [NBLK - 1] + list(range(NBLK - 1))
            cnts = [None] * NBLK
            cts = [None] * NBLK
            for cc in range(CC):
                for oi, j in enumerate(order):
                    pj = min(P, ROWS - j * P)
                    if cc == 0:
                        cnts[j], cts[j] = emit_countsT(j, pj)
                        if oi == 0:
                            # order the W build after the first histogram on
                            # DVE: interleaving would inflate the first
                            # transpose's DVE-op-count semaphore (~3us ramp)
                            w0a = build_w(0, 0, MW // 2)
                            _add_dep_helper(
                                w0a.ins,
                                state["hist_last"].ins,
                                sync=False,
                                reason="keep first hist ops contiguous",
                            )
                        elif oi in (5, 8, 11):
                            build_w(oi // 3, 0, MW)
                        emit_half(cc, j, pj, cts[j], 0)
                        if oi == 0:
                            w0b = build_w(0, MW // 2, MW)
                            _add_dep_helper(
                                w0b.ins, w0a.ins, sync=False,
                                reason="W0 halves in order",
                            )
                        emit_half(cc, j, pj, cts[j], 1)
                        for c, q in EXTRA0.get(oi, ()):
                            build_embrep(c, q)
                    elif oi in DVE_SET[cc]:
                        # direct broadcast multiply on DVE, no PSUM
                        ot = wpool.tile([P, CW], F32, tag="ot")
                        nc.vector.tensor_tensor(
                            out=ot[:pj].rearrange("p (m d) -> p m d", d=D),
                            in0=cnts[j][
                                :pj, cc * MW : (cc + 1) * MW, None
                            ].broadcast_to([pj, MW, D]),
                            in1=Et[cc][:pj].rearrange("p (m d) -> p m d", d=D),
                            op=mybir.AluOpType.mult,
                        )
                        nc.sync.dma_start(
                            out=out[j * P : j * P + pj, cc * CW : (cc + 1) * CW],
                            in_=ot[:pj],
                        )
                    else:
                        ot = wpool.tile([P, CW], F32, tag="ot")
                        for h in range(2):
                            emit_half(cc, j, pj, cts[j], h, ot=ot)
                        nc.sync.dma_start(
                            out=out[j * P : j * P + pj, cc * CW : (cc + 1) * CW],
                            in_=ot[:pj],
                        )
                        if cc == 1:
                            for c, q in EXTRA1.get(oi, ()):
                                build_embrep(c, q)

    nc.finalize()
    return nc


def _get_nc():
    if "nc" not in _NC_CACHE:
        _NC_CACHE["nc"] = _build_nc()
    return _NC_CACHE["nc"]


def _prepare_in_maps(concepts, emb_table):
    concepts = np.asarray(concepts)
    emb = np.asarray(emb_table, dtype=np.float32)

    # per-core index shards, padded to NBLK*P rows, laid out [P, NBLK*K]
    conc = concepts.reshape(NCORES, ROWS, K).astype(np.float32)
    idx_pad = np.full((NCORES, NBLK * P, K), float(N), dtype=np.float32)
    idx_pad[:, :ROWS] = conc
    # [core, NBLK, P, K] -> [core, P, NBLK*K]
    idx_dev = idx_pad.reshape(NCORES, NBLK, P, K).transpose(0, 2, 1, 3).reshape(
        NCORES, P, NBLK * K
    )

    cst = np.empty((NCORES, P, C_TOT), dtype=np.float32)
    cst[:, :, C_IOTA : C_IOTA + N] = np.arange(N, dtype=np.float32)
    cst[:, :, C_IDX : C_IDX + NBLK * K] = idx_dev
    cst[:, :, C_EMB : C_EMB + D] = emb
    cst[:, :, C_ID : C_ID + P] = np.eye(P, dtype=np.float32)
    cst = np.ascontiguousarray(cst)
    return [{"cst": cst[i]} for i in range(NCORES)]


def _run(concepts, emb_table, **spmd_kwargs):
    nc = _get_nc()
    in_maps = _prepare_in_maps(concepts, emb_table)
    res = run_bass_kernel_spmd(nc, in_maps, core_ids=list(range(NCORES)), **spmd_kwargs)
    out = np.concatenate(
        [res.results[i]["out"].reshape(B_LOC, S, N, D) for i in range(NCORES)],
        axis=0,
    )
    return out, res


def kernel(concepts, emb_table):
    out, _ = _run(concepts, emb_table)
    return out
